# revision 13
# baseline (speedup 1.0000x reference)
"""Trainium2 Bass kernel for nn_AccuracyMetricLoss.

Computes mean over 200000 days of per-day scores:
    denom = max(t, 0.2*cap);  rel_sq = ((t-p)/denom)^2
    score_d = (1 - sqrt(mean_96(rel_sq))) * 100;  out = mean_d(score_d)

Sharding: day axis split evenly across 8 NeuronCores (25000 days/core).

Per-core pipeline: inputs stream in as large DRAM-sequential chunks (all
on the SP HWDGE ring; all chunks stay resident in SBUF so loads are never
gated on compute). Per chunk:
    ACT:  q = t^-1/2  (Abs_reciprocal_sqrt)
    PE :  d = I.t + (-I).p  accumulated in PSUM (f32r matmuls, 1 cyc/row)
    DVE:  custom fused op  s = cumsum(d^2 * min(q, thresh^-1/2)^4)
          per <=5-day PSUM-bank slice (PSUM in0, SBUF in1/out)
    ACT:  copy strided per-day prefix samples s[:, 95::96] into acc
Output stored in two pieces (bulk early, tiny store at the end).
Host: difference the per-slice prefix samples -> per-day sums, then
sqrt/score/mean in f64.

The PE offload keeps DVE to one pass over the data (~0.1us/day), well
under the DMA stream rate (~0.23us/day), so the kernel is DMA-bound and
the compute tail after the last byte lands is minimal.
"""
import os
import sys

sys.path.insert(0, "/opt/trn_rl_repo")

import numpy as np

import concourse.bacc as bacc
import concourse.mybir as mybir
from concourse.bass_utils import run_bass_kernel_spmd
from concourse.tile import TileContext

from concourse.dve_ops import DveOp, OPS, CUSTOM_DVE_SPECS, _SUB_OPCODE_FOR_NAME
from concourse.dve_spec import Spec, Src0, Src1, C0, AluOp, sq, minn, scan, lower
from concourse.dve_uop import DveOpSpec

# ---------------- problem constants (hardcoded) ---------------- #
CAP = (300 + 400 + 900) / 300 / 1000 * 300400.0  # 1602.1333...
THRESH = np.float32(0.2) * np.float32(CAP)
CQ = float(np.float64(THRESH) ** -0.5)  # clamp for q = t^-1/2  (q^4 = 1/t^2)
T = 96
N_DAYS = 200000
N_CORES = 8
DAYS_PER_CORE = N_DAYS // N_CORES  # 25000
P = 128
# DMA chunks: contiguous flat ranges reshaped [rows, days_per_row*96].
# (rows, days_per_row, [compute slice day-widths])
# Large DMA chunks keep descriptors big (DMA efficiency); compute slices
# are <=5 days so each slice's difference tile fits one PSUM bank.  The
# tail chunks shrink so little compute remains after the last byte lands.
_SCHED = [12, 20, 30, 40, 40, 30, 12, 8, 2, 1]
assert sum(_SCHED) == 195


def _slices(d):
    out = []
    while d > 0:
        s = min(5, d)
        out.append(s)
        d -= s
    return out


CHUNKS = [(128, d, _slices(d)) for d in _SCHED] + [(40, 1, [1])]
assert sum(r * c for r, c, _ in CHUNKS) == DAYS_PER_CORE
for _r, _c, _s in CHUNKS:
    assert sum(_s) == _c
ACC_COLS = sum(sum(s) for _, _, s in CHUNKS)  # 196
SLICE_FD = 5 * T  # one PSUM bank (480 of 512 f32)
RING = 4  # lt/st ring depth (slices)


def _register_clamp_sq_scan():
    # out = cumsum(in0^2 * min(in1, s0)^4): in0 = t-p, in1 = t^-1/2,
    # s0 = thresh^-1/2, so min(in1,s0)^4 = 1/max(t,thresh)^2
    name = "CLAMP4_SQ_SCAN_ANT"
    for op in OPS:
        if op.name == name:
            return op

    qc = minn(Src1, C0)
    body = scan(AluOp.ADD, sq(Src0) * sq(sq(qc)))

    def _ref(in0, in1, s0, s1, imm2):
        x = np.asarray(in0, np.float32)
        r = np.asarray(in1, np.float32).reshape(x.shape[0], -1)
        c = s0 if isinstance(s0, float) else np.asarray(s0, np.float32).reshape(-1, 1)
        b = (x.reshape(x.shape[0], -1) ** 2) * np.minimum(r, c) ** 4
        out = np.cumsum(b.astype(np.float32), axis=-1, dtype=np.float32)
        return out.reshape(in0.shape)

    spec = Spec(body=body, reference=_ref)
    row = 1 + len(OPS)
    assert row < 0x20
    _SUB_OPCODE_FOR_NAME[name] = row
    shas = {}
    for ver in ("v3", "v4"):
        u = lower(spec, ver=ver)
        shas[ver] = DveOpSpec(name=name, opcode=row, uops=u, rd1_en=True).sha(ver)
    op = DveOp(name, spec, subdim=False, uops_sha=shas)
    OPS.append(op)
    CUSTOM_DVE_SPECS[name] = spec
    return op


_nc_cache = {}


def _build_nc():
    if "nc" in _nc_cache:
        return _nc_cache["nc"]
    clamp_sq_scan = _register_clamp_sq_scan()

    nc = bacc.Bacc("TRN2")
    n_elem = DAYS_PER_CORE * T
    t_in = nc.dram_tensor("t_in", [n_elem], mybir.dt.float32, kind="ExternalInput")
    p_in = nc.dram_tensor("p_in", [n_elem], mybir.dt.float32, kind="ExternalInput")
    eye_in = nc.dram_tensor("eye2", [P, 2 * P], mybir.dt.float32, kind="ExternalInput")
    out = nc.dram_tensor("out", [P, ACC_COLS], mybir.dt.float32, kind="ExternalOutput")

    f32r = mybir.dt.float32r
    with TileContext(nc) as tc:
        with (
            tc.tile_pool(name="tp", bufs=1) as tp,
            tc.tile_pool(name="pp", bufs=1) as pp,
            tc.tile_pool(name="lp", bufs=RING) as lp,
            tc.tile_pool(name="sp", bufs=RING) as sp,
            tc.tile_pool(name="accp", bufs=1) as accp,
            tc.tile_pool(name="eyep", bufs=1) as eyep,
            tc.psum_pool(name="dp", bufs=8) as dp,
        ):
            acc = accp.tile([P, ACC_COLS], mybir.dt.float32)
            eye = eyep.tile([P, 2 * P], mybir.dt.float32)
            # loads land tagged f32r (same bytes) so the BIR verifier
            # accepts them as fp32r-matmul operands; the PE truncates the
            # mantissa in hardware either way
            nc.sync.dma_start(out=eye[:].bitcast(f32r), in_=eye_in[:].bitcast(f32r))
            # all chunks stay resident: loads never gated on compute;
            # single ring (SP), ordered t0,p0,t1,p1,... so early chunks
            # complete first and compute streams behind the loads
            t_tiles, p_tiles = [], []
            base = 0
            for ci, (rows, cdays, _) in enumerate(CHUNKS):
                fd = cdays * T
                t = tp.tile([P, fd], mybir.dt.float32, tag=f"t{ci}")
                p = pp.tile([P, fd], mybir.dt.float32, tag=f"p{ci}")
                n = rows * fd
                t_v = t_in[base : base + n].rearrange("(p f) -> p f", p=rows)
                p_v = p_in[base : base + n].rearrange("(p f) -> p f", p=rows)
                nc.sync.dma_start(out=t[:rows, :].bitcast(f32r), in_=t_v.bitcast(f32r))
                nc.sync.dma_start(out=p[:rows, :].bitcast(f32r), in_=p_v.bitcast(f32r))
                t_tiles.append(t)
                p_tiles.append(p)
                base += n
            # build the global slice list with pre-allocated ring tiles
            sl = []  # (rows, t_ap, p_ap, lt_tile, st_tile, d_tile, fd, acc_col, sdays)
            acc_col = 0
            for ci, (rows, cdays, slices) in enumerate(CHUNKS):
                off = 0
                for sdays in slices:
                    fd = sdays * T
                    cols = slice(off * T, off * T + fd)
                    sl.append(
                        (
                            rows,
                            t_tiles[ci][:rows, cols],
                            p_tiles[ci][:rows, cols],
                            lp.tile([P, SLICE_FD], mybir.dt.float32, tag="lt", name="lt"),
                            sp.tile([P, SLICE_FD], mybir.dt.float32, tag="st", name="st"),
                            dp.tile([P, 512], mybir.dt.float32, tag="d", name="d"),
                            fd,
                            acc_col,
                            sdays,
                        )
                    )
                    off += sdays
                    acc_col += sdays
            # PE: per chunk, all t-matmuls (weights +I) then all p-matmuls
            # (weights -I) so walrus can keep the weights loaded; the PSUM
            # bank ring (8 banks) decouples chunks
            k = 0
            for ci, (rows, cdays, slices) in enumerate(CHUNKS):
                group = sl[k : k + len(slices)]
                for rows_, ts, ps, lt, st, d, fd, _, _ in group:
                    nc.tensor.matmul(
                        d[:rows_, :fd],
                        eye[:rows_, 0:rows_].bitcast(f32r),
                        ts.bitcast(f32r),
                        start=True,
                        stop=False,
                        skip_group_check=True,
                    )
                for rows_, ts, ps, lt, st, d, fd, _, _ in group:
                    nc.tensor.matmul(
                        d[:rows_, :fd],
                        eye[:rows_, P : P + rows_].bitcast(f32r),
                        ps.bitcast(f32r),
                        start=False,
                        stop=True,
                        skip_group_check=True,
                    )
                k += len(slices)
            # DVE: one fused scan per slice (PSUM in0, SBUF in1/out)
            for rows, ts, ps, lt, st, d, fd, _, _ in sl:
                nc.vector._custom_dve(
                    clamp_sq_scan,
                    out=st[:rows, :fd],
                    in0=d[:rows, :fd],
                    in1=lt[:rows, :fd],
                    s0=CQ,
                )
            # ACT: per-slice rsqrt, with the per-day sample copies woven in
            # RING slices behind so a copy (gated on its scan) never
            # head-of-line-blocks the next rsqrt (gated only on its load)
            for k in range(len(sl) + RING):
                if k < len(sl):
                    rows, ts, ps, lt, st, d, fd, _, _ = sl[k]
                    nc.scalar.activation(
                        lt[:rows, :fd],
                        ts,
                        mybir.ActivationFunctionType.Abs_reciprocal_sqrt,
                    )
                if k >= RING:
                    rows, ts, ps, lt, st, d, fd, acol, sdays = sl[k - RING]
                    samples = st[:rows, :fd].rearrange("p (c n) -> p c n", n=T)[
                        :, :, 95
                    ]
                    nc.scalar.copy(acc[:rows, acol : acol + sdays], samples)
            # split the result store: the bulk goes out while the last
            # chunks still compute; only a tiny store remains at the end
            split = ACC_COLS - (sum(_SCHED[-2:]) + 1)
            nc.sync.dma_start(out=out[:, :split], in_=acc[:, :split])
            nc.sync.dma_start(out=out[:, split:], in_=acc[:, split:])
    nc.finalize()
    _nc_cache["nc"] = nc
    return nc


_last_results = None
_EYE2 = None


def kernel(pred: np.ndarray, true: np.ndarray) -> np.ndarray:
    global _last_results, _EYE2
    nc = _build_nc()

    if _EYE2 is None:
        _EYE2 = np.concatenate(
            [np.eye(P, dtype=np.float32), -np.eye(P, dtype=np.float32)], axis=1
        )
        _EYE2 = np.ascontiguousarray(_EYE2)

    n_elem = DAYS_PER_CORE * T
    pred = np.ascontiguousarray(pred, dtype=np.float32)
    true = np.ascontiguousarray(true, dtype=np.float32)
    in_maps = [
        {
            "t_in": true[k * n_elem : (k + 1) * n_elem],
            "p_in": pred[k * n_elem : (k + 1) * n_elem],
            "eye2": _EYE2,
        }
        for k in range(N_CORES)
    ]

    trace = False
    if os.environ.get("BASS_TRACE"):
        try:  # tracing needs the axon NTFF hook; never crash without it
            import antenv.axon_hooks  # noqa: F401

            trace = True
        except ImportError:
            pass
    res = run_bass_kernel_spmd(nc, in_maps, list(range(N_CORES)), trace=trace)
    _last_results = res

    # host-side tail: prefix samples -> day sums -> scores -> mean
    total = 0.0
    for k in range(N_CORES):
        A = res.results[k]["out"].astype(np.float64)  # [128, ACC_COLS]
        acc_col = 0
        for rows, cdays, slices in CHUNKS:
            for sdays in slices:
                S = A[:rows, acc_col : acc_col + sdays]
                u = S.copy()
                u[:, 1:] -= S[:, :-1]  # per-day sums of rel_sq
                np.maximum(u, 0.0, out=u)  # guard sqrt against diff rounding
                scores = (1.0 - np.sqrt(u / T)) * 100.0
                total += scores.sum()
                acc_col += sdays
    return np.float32(total / N_DAYS)
